# revision 17
# baseline (speedup 1.0000x reference)
"""Trainium2 Bass kernel for GQA MultiHeadAttention with RoPE (v5).

Shapes (hardcoded): x (2,2048,1024), Wq (1024,1024), Wk/Wv (1024,256),
Wo (1024,1024). 16 q-heads, 4 kv-heads, head_dim 64.

Sharding: 8 cores = batch (2) x kv-group (4). Core i handles b=i//4,
g=i%4, q-heads {g, 4+g, 8+g, 12+g} (jnp.tile GQA mapping), kv-head g.
Each core emits a partial Y^T (1024,2048) in bf16; the host sums the 4
group partials per batch in f32 and transposes.

Faithful to the reference's multiplicative tril mask before softmax:
  P = exp(mask * (Q K^T) * D**-0.5)   (masked entries = exp(0) = 1)
  out = (P @ V_aug) / Z,  Z carried in V_aug's ones column.

v5 structure vs v4:
- Chunks ASCEND (0..3) with per-chunk incremental KV projection/rope,
  so the first exp fires ~8us in instead of ~58us, and the tail is one
  small Y-proj drain instead of a full chunk.
- Unified O path: one [V|1] (M=65) stationary serves both heads of a
  pair -- two consecutive matmuls (no LDWEIGHTS re-pay) into separate
  PSUM banks opsE/opsO; the old M=128 vaugO layout (and its memset +
  16 SBUF copies) is gone.  Odd head's normalize writes ostk rows
  64-127 from base-0 operands (cross-partition-base DVE, HW-verified).
- Diagonal-tile masking moved post-exp: a bf16 0/1 tril multiply on pq
  (one op covers both heads via a strided [128,2,128] view) replaces
  the f32 PSUM -1e9 adds.
- In-chunk exp(0)=1 corrections (U-mask matmuls) and the beyond-chunk
  analytic V-suffix/count terms are unchanged from v4.
"""

import os
import numpy as np
import ml_dtypes

import concourse.bass as bass
import concourse.mybir as mybir
import concourse.tile as tile
from concourse.masks import make_identity
from concourse.bass_utils import run_bass_kernel_spmd

F32 = mybir.dt.float32
DTMM = mybir.dt.bfloat16          # matmul operand dtype
NPMM = ml_dtypes.bfloat16
EXP = mybir.ActivationFunctionType.Exp

B, T, C = 2, 2048, 1024
NH, NKV, D = 16, 4, 64
HG = NH // NKV            # 4 q-heads per kv-group
NQ = 512                  # tq chunk width
NCH = T // NQ             # 4 chunks
NKT = T // 128            # 16 tk tiles
SCALE = D ** -0.5


def _split_waits(nc, max_waits=1):
    """This walrus build accepts only one immediate sem-wait per
    instruction; move extras onto preceding same-engine NoOps."""
    for f in nc.m.functions:
        for blk in f.blocks:
            new_insts = []
            for ins in blk.instructions:
                si = ins.sync_info
                if si is not None and len(si.on_wait) > max_waits:
                    waits = list(si.on_wait)
                    extra, keep = waits[:-max_waits], waits[-max_waits:]
                    k = 0
                    while extra:
                        chunk, extra = extra[:max_waits], extra[max_waits:]
                        nop = mybir.InstNoOp(name=f"{ins.name}-ws{k}", ins=[], outs=[])
                        nop.engine = ins.engine
                        nop.sync_info = mybir.SyncInfo(on_wait=chunk, on_update=[])
                        new_insts.append(nop)
                        k += 1
                    si.on_wait = keep
                new_insts.append(ins)
            blk.instructions[:] = new_insts


def _half_swap(nc, dst, src, base, eng=None):
    """dst rows [base:base+64] = src rows [base+32:base+64],[base:base+32]."""
    eng = eng or nc.gpsimd
    eng.dma_start(out=dst[base:base + 32, :], in_=src[base + 32:base + 64, :])
    eng.dma_start(out=dst[base + 32:base + 64, :], in_=src[base:base + 32, :])


def _emit(nc, tc, ctx, xT, wq, wkv, wo, ctab, stab, m2tab, umtab, bseltab, yT):
    # ---------- whole-kernel SBUF ----------
    poolW = ctx.enter_context(tc.tile_pool(name="poolW", bufs=1))
    qrot = [poolW.tile([128, T], DTMM, tag=f"qrot{p}", name=f"qrot{p}")
            for p in range(2)]
    krot2 = poolW.tile([128, T], DTMM)      # K + dup; V parked in 64-127 per chunk
    vaug = poolW.tile([128, NKT * 65], DTMM)    # per kt: [V | 1]
    m2 = poolW.tile([128, 2 * 128], DTMM)   # 0/1 tril keep-mask, dup'd per head
    umaskb = poolW.tile([128, 4 * NQ], DTMM)   # strict-upper ones per j
    ostk = [poolW.tile([128, T], DTMM, tag=f"ostk{p}", name=f"ostk{p}")
            for p in range(2)]              # rows 0-63 head 2p, 64-127 head 2p+1
    sfst = poolW.tile([128, 4], F32)        # suffix sums of V, stacked both halves
    jnk = poolW.tile([128, NQ + 64], DTMM)  # warmup operands (zeros)
    bsel = poolW.tile([2, 128], DTMM)       # 1/Z broadcast selector
    smallf = poolW.tile([128, 64], F32)     # identity scratch
    smallr = poolW.tile([128, 64], DTMM)
    IDR = smallr[:, 0:64]

    nc.gpsimd.memset(jnk[:], 0.0)
    nc.gpsimd.memset(smallf[:], 0.0)
    make_identity(nc, smallf[0:64, :], nomemset=True)
    nc.gpsimd.dma_start(out=smallf[64:128, :], in_=smallf[0:64, :])
    with nc.allow_low_precision(reason="bf16 constants"):
        nc.vector.tensor_copy(smallr[:], smallf[:])
    vaug3 = vaug[:].rearrange("p (k c) -> p k c", c=65)
    nc.vector.memset(vaug3[:, :, 64:65], 1.0)

    # ---------- single-phase pools (no release barriers) ----------
    poolA = ctx.enter_context(tc.tile_pool(name="poolA", bufs=1))
    stg = ctx.enter_context(tc.tile_pool(name="stg", bufs=3))
    poolB = ctx.enter_context(tc.tile_pool(name="poolB", bufs=1))
    ps = ctx.enter_context(tc.tile_pool(name="ps", bufs=1, space="PSUM"))

    xtr = poolA.tile([128, 8 * T], DTMM)    # x^T, all 8 row-blocks
    wqr = poolA.tile([128, 8 * 256], DTMM)
    wkvr = poolA.tile([128, 8 * 128], DTMM)
    cost = poolA.tile([128, T], DTMM)
    sint = poolA.tile([128, T], DTMM)
    redc = poolA.tile([128, 8], F32)        # per-chunk V sums (rows 64-127)
    wor = poolB.tile([128, 2 * C], DTMM)

    # PE warmup on dependency-light zero tiles: un-throttle HAM early and
    # keep it warm across the input-DMA window (idle >3.4us re-throttles).
    for w in range(26):
        wps = ps.tile([128, NQ], F32, tag="aux", bufs=2, name="wps")
        nc.tensor.matmul(wps[0:64, :], jnk[0:64, 0:64],
                         jnk[0:64, 64:64 + NQ], start=True, stop=True)

    # consolidated input loads, ordered so chunk-0 KV/Q start earliest
    nc.sync.dma_start(
        out=wkvr[:].rearrange("p (i c) -> p i c", i=8),
        in_=wkv.rearrange("(i p) c -> p i c", i=8))
    xtr3 = xtr[:].rearrange("p (i t) -> p i t", i=8)
    xT3 = xT.rearrange("(i p) t -> p i t", i=8)
    nc.sync.dma_start(out=xtr3[:, :, 0:NQ], in_=xT3[:, :, 0:NQ])
    nc.sync.dma_start(
        out=wqr[:].rearrange("p (i c) -> p i c", i=8),
        in_=wq.rearrange("(i p) c -> p i c", i=8))
    nc.sync.dma_start(out=cost[:, 0:NQ], in_=ctab[:, 0:NQ])
    nc.sync.dma_start(out=sint[:, 0:NQ], in_=stab[:, 0:NQ])
    # small tables ride the gpsimd queue; x chunks + rope tables on sync
    nc.gpsimd.dma_start(out=m2[:], in_=m2tab[:])
    nc.gpsimd.dma_start(out=umaskb[:], in_=umtab[:])
    nc.gpsimd.dma_start(out=bsel[:], in_=bseltab[:])
    nc.gpsimd.dma_start(
        out=wor[:].rearrange("p (j c) -> p j c", j=2),
        in_=wo.rearrange("(j p) c -> p j c", j=2))
    for cc in range(1, NCH):
        cs = slice(cc * NQ, (cc + 1) * NQ)
        nc.sync.dma_start(out=xtr3[:, :, cs], in_=xT3[:, :, cs])
        nc.sync.dma_start(out=cost[:, cs], in_=ctab[:, cs])
        nc.sync.dma_start(out=sint[:, cs], in_=stab[:, cs])

    def emit_kv(c):
        """KV proj chunk c; K rope into krot2[0:64,sl] + dup to [64:128];
        V parked transiently in krot2[64:128,sl], consumed into vaug +
        per-chunk sum redc[:,c] before the dup overwrites it."""
        sl = slice(c * NQ, (c + 1) * NQ)
        kvps = ps.tile([128, NQ], F32, tag="aux", bufs=2, name="kvps")
        for i in range(8):
            nc.tensor.matmul(kvps[:], wkvr[:, i * 128:(i + 1) * 128],
                             xtr[:, i * T + c * NQ:i * T + (c + 1) * NQ],
                             start=(i == 0), stop=(i == 7))
            if i in (1, 3, 5):
                yield
        kcp = stg.tile([128, NQ], DTMM, tag="pcp")
        with nc.allow_low_precision(reason="bf16 KV evac"):
            nc.vector.tensor_copy(kcp[0:64, :], kvps[0:64, :])
            nc.vector.tensor_copy(krot2[64:128, sl], kvps[64:128, :])
        yield
        for j in range(4):
            kt = c * 4 + j
            vtp = ps.tile([128, 64], DTMM, tag="aux", bufs=2, name="vtp")
            with nc.allow_low_precision(reason="bf16 PE transpose of V"):
                nc.tensor.transpose(vtp[:],
                                    krot2[64:128, kt * 128:(kt + 1) * 128],
                                    IDR[64:128, :])
                nc.vector.tensor_copy(vaug3[:, kt, 0:64], vtp[:])
            if j in (0, 1, 2):
                yield
        # per-chunk V sum (for the beyond-chunk analytic term)
        nc.vector.tensor_reduce(redc[64:128, c:c + 1], krot2[64:128, sl],
                                axis=mybir.AxisListType.X,
                                op=mybir.AluOpType.add)
        yield
        swp = stg.tile([128, NQ], DTMM, tag="swp")
        _half_swap(nc, swp, kcp, 0)
        t1 = stg.tile([128, NQ], DTMM, tag="t1")
        with nc.allow_low_precision(reason="bf16 K rope"):
            nc.vector.tensor_mul(t1[0:64, :], kcp[0:64, :], cost[0:64, sl])
            nc.vector.tensor_mul(swp[0:64, :], swp[0:64, :], sint[0:64, sl])
            nc.vector.tensor_add(krot2[0:64, sl], t1[0:64, :], swp[0:64, :])
        yield
        nc.gpsimd.dma_start(out=krot2[64:128, sl], in_=krot2[0:64, sl])

    def finish_suffix():
        # sfst[:, c] = sum_{c' > c} chunk_sum[c']  (both partition halves)
        nc.vector.memset(redc[64:128, 7:8], 0.0)
        nc.vector.tensor_copy(sfst[64:128, 3:4], redc[64:128, 7:8])
        for c in (2, 1, 0):
            nc.vector.tensor_add(sfst[64:128, c:c + 1],
                                 sfst[64:128, c + 1:c + 2],
                                 redc[64:128, c + 1:c + 2])
        nc.gpsimd.dma_start(out=sfst[0:64, :], in_=sfst[64:128, :])

    def emit_qproj(c, p):
        sweng = nc.gpsimd if c == 0 else nc.sync
        sl = slice(c * NQ, (c + 1) * NQ)
        qps = ps.tile([128, NQ], F32, tag="aux", bufs=2, name="qps")
        for i in range(8):
            nc.tensor.matmul(
                qps[:], wqr[:, i * 256 + p * 128: i * 256 + (p + 1) * 128],
                xtr[:, i * T + c * NQ:i * T + (c + 1) * NQ],
                start=(i == 0), stop=(i == 7))
            if i in (1, 3, 5):
                yield
        qcp = stg.tile([128, NQ], DTMM, tag="pcp", name="qcp")
        with nc.allow_low_precision(reason="bf16 Q evac"):
            nc.vector.tensor_copy(qcp[:], qps[:])
        swp = stg.tile([128, NQ], DTMM, tag="swp", name="swp")
        _half_swap(nc, swp, qcp, 0, eng=sweng)
        _half_swap(nc, swp, qcp, 64, eng=sweng)
        t1 = stg.tile([128, NQ], DTMM, tag="t1", name="t1")
        with nc.allow_low_precision(reason="bf16 Q rope"):
            nc.vector.tensor_mul(t1[:], qcp[:], cost[:, sl])
            nc.vector.tensor_mul(swp[:], swp[:], sint[:, sl])
            nc.vector.tensor_add(qrot[p][:, sl], t1[:], swp[:])

    COPYF = mybir.ActivationFunctionType.Copy

    def emit_yproj_unit(c, j):
        csl = slice(c * NQ, (c + 1) * NQ)
        jsl = slice(j * 128, (j + 1) * 128)
        if c == NCH - 1 and j % 2 == 1:
            # sg banks are free once the last exp has read them
            ysg = ps.tile([128, 2 * NQ], F32, tag="sg", bufs=2, name="ysg")
            yps = ysg[:, 0:NQ]
        else:
            yps = ps.tile([128, NQ], F32, tag="aux", bufs=2, name="yps")
        for p in range(2):
            nc.tensor.matmul(yps, wor[:, p * C + j * 128:p * C + (j + 1) * 128],
                             ostk[p][:, csl],
                             start=(p == 0), stop=(p == 1))
        ytmp = poolB.tile([128, NQ], DTMM, tag="ytmp", bufs=3, name="ytmp")
        with nc.allow_low_precision(reason="bf16 Y out"):
            if c == NCH - 1 and j % 2 == 0:
                nc.scalar.activation(ytmp[:], yps, COPYF)
            else:
                nc.vector.tensor_copy(ytmp[:], yps)
        nc.sync.dma_start(out=yT[jsl, csl], in_=ytmp[:])
        yield

    # two fill queues: P (KV/Q for upcoming chunks, priority) and Y.
    fillsP = []
    fillsY = []
    cur_fill = [None]
    allow_y = [False]

    def pump():
        while True:
            if cur_fill[0] is None:
                if fillsP:
                    cur_fill[0] = fillsP.pop(0)
                elif fillsY and allow_y[0]:
                    cur_fill[0] = fillsY.pop(0)
                else:
                    return
            try:
                next(cur_fill[0])
                return
            except StopIteration:
                cur_fill[0] = None

    def force(gen):
        if cur_fill[0] is gen:
            cur_fill[0] = None
        if gen in fillsP:
            fillsP.remove(gen)
        for _ in gen:
            pass

    def drain_fills():
        allow_y[0] = True
        while cur_fill[0] is not None or fillsP or fillsY:
            pump()

    # ---------- prelude: chunk 0 KV + Q; queue the rest ----------
    kv_gens = [emit_kv(c) for c in range(NCH)]
    q_gens = [[emit_qproj(c, p) for p in range(2)] for c in range(NCH)]
    force(kv_gens[0])
    force(q_gens[0][0])
    fillsP.extend([q_gens[0][1], kv_gens[1], q_gens[1][0], q_gens[1][1],
                   kv_gens[2], q_gens[2][0], q_gens[2][1],
                   kv_gens[3], q_gens[3][0], q_gens[3][1]])
    suffix_done = [False]

    LA = 3
    for c in range(NCH):
        allow_y[0] = c >= 2
        csl = slice(c * NQ, (c + 1) * NQ)
        for p in range(2):
            force(kv_gens[c])
            force(q_gens[c][p])
            nkt = 4 * (c + 1)
            opsE = ps.tile([128, NQ], F32, tag="opsE", bufs=1, name="opsE")
            opsO = ps.tile([128, NQ], F32, tag="opsO", bufs=1, name="opsO")
            pqs = {}
            for idx in range(nkt + LA):
                if idx < nkt:
                    kt = idx
                    dlt = max(0, (kt - 4 * c)) * 128 if kt >= 4 * c else 0
                    sg = ps.tile([128, 2 * NQ], F32, tag="sg", bufs=2, name="sg")
                    nc.tensor.matmul(sg[:, dlt:NQ],
                                     krot2[0:64, kt * 128:(kt + 1) * 128],
                                     qrot[p][0:64, c * NQ + dlt:(c + 1) * NQ],
                                     start=True, stop=True)
                    nc.tensor.matmul(sg[:, NQ + dlt:2 * NQ],
                                     krot2[64:128, kt * 128:(kt + 1) * 128],
                                     qrot[p][64:128, c * NQ + dlt:(c + 1) * NQ],
                                     start=True, stop=True, tile_position=(64, 0))
                    pq = poolB.tile([128, 2 * NQ], DTMM, tag="pq", bufs=8,
                                    name="pq")
                    if dlt:
                        sgv = sg[:].rearrange("p (h q) -> p h q", h=2)
                        pqv = pq[:].rearrange("p (h q) -> p h q", h=2)
                        nc.scalar.activation(pqv[:, :, dlt:NQ],
                                             sgv[:, :, dlt:NQ], EXP, scale=SCALE)
                    else:
                        nc.scalar.activation(pq[:], sg[:], EXP, scale=SCALE)
                    if kt >= 4 * c:     # diag: zero the in-tile strict-upper
                        pqv = pq[:].rearrange("p (h q) -> p h q", h=2)
                        m2v = m2[:].rearrange("p (h q) -> p h q", h=2)
                        with nc.allow_low_precision(reason="bf16 tri mask"):
                            nc.vector.tensor_mul(pqv[:, :, dlt:dlt + 128],
                                                 pqv[:, :, dlt:dlt + 128],
                                                 m2v[:])
                    pqs[kt] = pq
                if idx == 2:
                    # U-mask matmuls: analytic exp(0)=1 corrections for the
                    # strictly-masked in-chunk region; ones col fixes Z counts.
                    # j descends so the start=True write covers the full width
                    # (keeps the sim's PSUM zero-tracking happy; same sum).
                    for j in (3, 2, 1, 0):
                        kt_d = 4 * c + j
                        un = 128 * (j + 1)
                        nc.tensor.matmul(
                            opsE[0:65, 0:un],
                            vaug3[:, kt_d, :],
                            umaskb[:, j * NQ:j * NQ + un],
                            start=(j == 3), stop=False)
                        nc.tensor.matmul(
                            opsO[0:65, 0:un],
                            vaug3[:, kt_d, :],
                            umaskb[:, j * NQ:j * NQ + un],
                            start=(j == 3), stop=False)
                if idx >= LA:
                    kt = idx - LA
                    dlt = max(0, (kt - 4 * c)) * 128 if kt >= 4 * c else 0
                    pq = pqs.pop(kt)
                    nc.tensor.matmul(opsE[0:65, dlt:NQ],
                                     vaug3[:, kt, :],
                                     pq[:, dlt:NQ],
                                     start=False, stop=(kt == nkt - 1))
                    nc.tensor.matmul(opsO[0:65, dlt:NQ],
                                     vaug3[:, kt, :],
                                     pq[:, NQ + dlt:2 * NQ],
                                     start=False, stop=(kt == nkt - 1))
                if idx % 2 == 1:
                    pump()
            # tail: evacuate O+Z, spread-reciprocal Z, broadcast, normalize
            ocpE = poolB.tile([128, NQ], DTMM, tag="ocpE", bufs=2, name="ocpE")
            ocpO = poolB.tile([128, NQ], DTMM, tag="ocpO", bufs=2, name="ocpO")
            with nc.allow_low_precision(reason="bf16 O evac"):
                nc.vector.tensor_copy(ocpE[0:65, :], opsE[0:65])
                nc.vector.tensor_copy(ocpO[0:65, :], opsO[0:65])
            pump()
            zsp = poolB.tile([128, 16], F32, tag="zsp", bufs=2, name="zsp")
            nc.gpsimd.dma_start(
                out=zsp[:, 0:4],
                in_=ocpE[64:65, :].rearrange("p (a b) -> p a b", b=4))
            nc.gpsimd.dma_start(
                out=zsp[:, 4:8],
                in_=ocpO[64:65, :].rearrange("p (a b) -> p a b", b=4))
            cnt = float(T - (c + 1) * NQ)
            nc.vector.tensor_scalar_add(zsp[:, 8:16], zsp[:, 0:8], cnt)
            nc.vector.reciprocal(zsp[:, 0:8], zsp[:, 8:16])
            rz2 = poolB.tile([2, NQ], DTMM, tag="rz2", bufs=2, name="rz2")
            nc.gpsimd.dma_start(
                out=rz2[0:1, :].rearrange("p (a b) -> p a b", b=4),
                in_=zsp[:, 0:4])
            nc.gpsimd.dma_start(
                out=rz2[1:2, :].rearrange("p (a b) -> p a b", b=4),
                in_=zsp[:, 4:8])
            if not suffix_done[0]:
                # first tail: every chunk's V sum must exist for sfst
                for cc in range(1, NCH):
                    force(kv_gens[cc])
                finish_suffix()
                suffix_done[0] = True
            pump()
            # reuse the opsE bank (already evacuated) so the 1/Z broadcast
            # does not enter the aux rotation used by projection fills
            rzp = ps.tile([128, NQ], F32, tag="opsE", bufs=1, name="rzp")
            nc.tensor.matmul(rzp[:], bsel[:], rz2[:], start=True, stop=True)
            with nc.allow_low_precision(reason="bf16 normalized O"):
                nc.vector.scalar_tensor_tensor(
                    ostk[p][0:64, csl], ocpE[0:64, :], sfst[0:64, c:c + 1],
                    rzp[0:64, :], op0=mybir.AluOpType.add,
                    op1=mybir.AluOpType.mult)
                nc.vector.scalar_tensor_tensor(
                    ostk[p][64:128, csl], ocpO[0:64, :], sfst[0:64, c:c + 1],
                    rzp[64:128, :], op0=mybir.AluOpType.add,
                    op1=mybir.AluOpType.mult)
        for j in range(8):
            fillsY.append(emit_yproj_unit(c, j))
    drain_fills()


def _build(nrep=1, split=True):
    from contextlib import ExitStack
    nc = bass.Bass()
    xT = nc.declare_dram_parameter("xT", [C, T], DTMM, isOutput=False)
    wq = nc.declare_dram_parameter("wq", [C, HG * D], DTMM, isOutput=False)
    wkv = nc.declare_dram_parameter("wkv", [C, 2 * D], DTMM, isOutput=False)
    wo = nc.declare_dram_parameter("wo", [HG * D, C], DTMM, isOutput=False)
    ctab = nc.declare_dram_parameter("ctab", [128, T], DTMM, isOutput=False)
    stab = nc.declare_dram_parameter("stab", [128, T], DTMM, isOutput=False)
    m2tab = nc.declare_dram_parameter("m2tab", [128, 256], DTMM, isOutput=False)
    umtab = nc.declare_dram_parameter("umtab", [128, 4 * NQ], DTMM,
                                      isOutput=False)
    bseltab = nc.declare_dram_parameter("bseltab", [2, 128], DTMM,
                                        isOutput=False)
    yT = nc.declare_dram_parameter("yT", [C, T], DTMM, isOutput=True)

    with tile.TileContext(nc) as tc:
        for _ in range(nrep):
            with ExitStack() as ctx:
                _emit(nc, tc, ctx, xT, wq, wkv, wo, ctab, stab, m2tab, umtab,
                      bseltab, yT)
    if split:
        _split_waits(nc)
    return nc


def _host_inputs(x, Wq, Wk, Wv, Wo):
    perm = np.concatenate([np.arange(0, D, 2), np.arange(1, D, 2)])  # even-first
    inv_freq = 1.0 / (10000.0 ** (np.arange(0, D, 2, dtype=np.float64) / D))
    ang = np.arange(T, dtype=np.float64)[:, None] * inv_freq[None, :]
    cos = np.cos(ang).astype(np.float32).T      # (32, T)
    sin = np.sin(ang).astype(np.float32).T
    ctab = np.ascontiguousarray(np.tile(cos, (4, 1)).astype(NPMM))    # (128, T)
    stab = np.ascontiguousarray(
        np.concatenate([-sin, sin, -sin, sin], 0).astype(NPMM))
    pcol = np.arange(128)[:, None]
    t128 = np.arange(128)[None, :]
    tril01 = (pcol <= t128).astype(NPMM)
    m2tab = np.ascontiguousarray(np.concatenate([tril01, tril01], axis=1))
    f = np.arange(NQ)[None, :]
    umtab = np.ascontiguousarray(np.concatenate(
        [(pcol + i * 128 > f).astype(NPMM) for i in range(4)], axis=1))
    bseltab = np.zeros((2, 128), dtype=NPMM)
    bseltab[0, 0:64] = 1.0
    bseltab[1, 64:128] = 1.0

    xTb = [np.ascontiguousarray(x[b].T.astype(NPMM)) for b in range(B)]
    maps = []
    for core in range(8):
        b, g = core // 4, core % 4
        heads = [g + NKV * k for k in range(HG)]
        wq_cols = np.concatenate([h * D + perm for h in heads])
        wq_g = np.ascontiguousarray(Wq[:, wq_cols].astype(NPMM))
        wkv_g = np.ascontiguousarray(np.concatenate(
            [Wk[:, g * D + perm], Wv[:, g * D:(g + 1) * D]], axis=1).astype(NPMM))
        wo_rows = np.concatenate([np.arange(h * D, (h + 1) * D) for h in heads])
        wo_g = np.ascontiguousarray(Wo[wo_rows, :].astype(NPMM))
        maps.append({"xT": xTb[b], "wq": wq_g, "wkv": wkv_g, "wo": wo_g,
                     "ctab": ctab, "stab": stab, "m2tab": m2tab, "umtab": umtab,
                     "bseltab": bseltab})
    return maps


_CACHE = {}


def kernel(x, Wq, Wk, Wv, Wo):
    if "nc" not in _CACHE:
        _CACHE["nc"] = _build()
    nc = _CACHE["nc"]
    maps = _host_inputs(np.asarray(x, np.float32), np.asarray(Wq, np.float32),
                        np.asarray(Wk, np.float32), np.asarray(Wv, np.float32),
                        np.asarray(Wo, np.float32))
    trace = bool(int(os.environ.get("BASSKERNEL_TRACE", "0")))
    res = run_bass_kernel_spmd(nc, maps, list(range(8)), trace=trace)
    if trace and res.exec_time_ns is not None:
        print(f"HW exec time: {res.exec_time_ns} ns")
    out = np.zeros((B, T, C), dtype=np.float32)
    for core in range(8):
        out[core // 4] += res.results[core]["yT"].astype(np.float32).T
    return out


# revision 18
# speedup vs baseline: 1.0453x; 1.0453x over previous
"""Trainium2 Bass kernel for GQA MultiHeadAttention with RoPE (v5).

Shapes (hardcoded): x (2,2048,1024), Wq (1024,1024), Wk/Wv (1024,256),
Wo (1024,1024). 16 q-heads, 4 kv-heads, head_dim 64.

Sharding: 8 cores = batch (2) x kv-group (4). Core i handles b=i//4,
g=i%4, q-heads {g, 4+g, 8+g, 12+g} (jnp.tile GQA mapping), kv-head g.
Each core emits a partial Y^T (1024,2048) in bf16; the host sums the 4
group partials per batch in f32 and transposes.

Faithful to the reference's multiplicative tril mask before softmax:
  P = exp(mask * (Q K^T) * D**-0.5)   (masked entries = exp(0) = 1)
  out = (P @ V_aug) / Z,  Z carried in V_aug's ones column.

v5 structure vs v4:
- Chunks ASCEND (0..3) with per-chunk incremental KV projection/rope,
  so the first exp fires ~8us in instead of ~58us, and the tail is one
  small Y-proj drain instead of a full chunk.
- Unified O path: one [V|1] (M=65) stationary serves both heads of a
  pair -- two consecutive matmuls (no LDWEIGHTS re-pay) into separate
  PSUM banks opsE/opsO; the old M=128 vaugO layout (and its memset +
  16 SBUF copies) is gone.  Odd head's normalize writes ostk rows
  64-127 from base-0 operands (cross-partition-base DVE, HW-verified).
- Diagonal-tile masking moved post-exp: a bf16 0/1 tril multiply on pq
  (one op covers both heads via a strided [128,2,128] view) replaces
  the f32 PSUM -1e9 adds.
- In-chunk exp(0)=1 corrections (U-mask matmuls) and the beyond-chunk
  analytic V-suffix/count terms are unchanged from v4.
"""

import os
import numpy as np
import ml_dtypes

import concourse.bass as bass
import concourse.mybir as mybir
import concourse.tile as tile
from concourse.masks import make_identity
from concourse.bass_utils import run_bass_kernel_spmd

F32 = mybir.dt.float32
DTMM = mybir.dt.bfloat16          # matmul operand dtype
NPMM = ml_dtypes.bfloat16
EXP = mybir.ActivationFunctionType.Exp

B, T, C = 2, 2048, 1024
NH, NKV, D = 16, 4, 64
HG = NH // NKV            # 4 q-heads per kv-group
NQ = 512                  # tq chunk width
NCH = T // NQ             # 4 chunks
NKT = T // 128            # 16 tk tiles
SCALE = D ** -0.5


def _split_waits(nc, max_waits=1):
    """This walrus build accepts only one immediate sem-wait per
    instruction; move extras onto preceding same-engine NoOps."""
    for f in nc.m.functions:
        for blk in f.blocks:
            new_insts = []
            for ins in blk.instructions:
                si = ins.sync_info
                if si is not None and len(si.on_wait) > max_waits:
                    waits = list(si.on_wait)
                    extra, keep = waits[:-max_waits], waits[-max_waits:]
                    k = 0
                    while extra:
                        chunk, extra = extra[:max_waits], extra[max_waits:]
                        nop = mybir.InstNoOp(name=f"{ins.name}-ws{k}", ins=[], outs=[])
                        nop.engine = ins.engine
                        nop.sync_info = mybir.SyncInfo(on_wait=chunk, on_update=[])
                        new_insts.append(nop)
                        k += 1
                    si.on_wait = keep
                new_insts.append(ins)
            blk.instructions[:] = new_insts


def _half_swap(nc, dst, src, base, eng=None):
    """dst rows [base:base+64] = src rows [base+32:base+64],[base:base+32]."""
    eng = eng or nc.gpsimd
    eng.dma_start(out=dst[base:base + 32, :], in_=src[base + 32:base + 64, :])
    eng.dma_start(out=dst[base + 32:base + 64, :], in_=src[base:base + 32, :])


def _emit(nc, tc, ctx, xT, wq, wkv, wo, ctab, stab, m2tab, umtab, bseltab, yT):
    # ---------- whole-kernel SBUF ----------
    poolW = ctx.enter_context(tc.tile_pool(name="poolW", bufs=1))
    qrot = [poolW.tile([128, T], DTMM, tag=f"qrot{p}", name=f"qrot{p}")
            for p in range(2)]
    krot2 = poolW.tile([128, T], DTMM)      # K + dup; V parked in 64-127 per chunk
    vaug = poolW.tile([128, NKT * 65], DTMM)    # per kt: [V | 1]
    m2 = poolW.tile([128, 2 * 128], DTMM)   # 0/1 tril keep-mask, dup'd per head
    umaskb = poolW.tile([128, 4 * NQ], DTMM)   # strict-upper ones per j
    ostk = [poolW.tile([128, T], DTMM, tag=f"ostk{p}", name=f"ostk{p}")
            for p in range(2)]              # rows 0-63 head 2p, 64-127 head 2p+1
    sfst = poolW.tile([128, 4], F32)        # suffix sums of V, stacked both halves
    jnk = poolW.tile([128, NQ + 64], DTMM)  # warmup operands (zeros)
    bsel = poolW.tile([2, 128], DTMM)       # 1/Z broadcast selector
    smallf = poolW.tile([128, 64], F32)     # identity scratch
    smallr = poolW.tile([128, 64], DTMM)
    IDR = smallr[:, 0:64]

    nc.gpsimd.memset(jnk[:], 0.0)
    nc.gpsimd.memset(smallf[:], 0.0)
    make_identity(nc, smallf[0:64, :], nomemset=True)
    nc.gpsimd.dma_start(out=smallf[64:128, :], in_=smallf[0:64, :])
    with nc.allow_low_precision(reason="bf16 constants"):
        nc.vector.tensor_copy(smallr[:], smallf[:])
    vaug3 = vaug[:].rearrange("p (k c) -> p k c", c=65)
    nc.vector.memset(vaug3[:, :, 64:65], 1.0)

    # ---------- single-phase pools (no release barriers) ----------
    poolA = ctx.enter_context(tc.tile_pool(name="poolA", bufs=1))
    stg = ctx.enter_context(tc.tile_pool(name="stg", bufs=3))
    poolB = ctx.enter_context(tc.tile_pool(name="poolB", bufs=1))
    ps = ctx.enter_context(tc.tile_pool(name="ps", bufs=1, space="PSUM"))

    xtr = poolA.tile([128, 8 * T], DTMM)    # x^T, all 8 row-blocks
    wqr = poolA.tile([128, 8 * 256], DTMM)
    wkvr = poolA.tile([128, 8 * 128], DTMM)
    cost = poolA.tile([128, T], DTMM)
    sint = poolA.tile([128, T], DTMM)
    redc = poolA.tile([128, 8], F32)        # per-chunk V sums (rows 64-127)
    wor = poolB.tile([128, 2 * C], DTMM)

    # PE warmup on dependency-light zero tiles: un-throttle HAM early and
    # keep it warm across the input-DMA window (idle >3.4us re-throttles).
    for w in range(26):
        wps = ps.tile([128, NQ], F32, tag="aux", bufs=2, name="wps")
        nc.tensor.matmul(wps[0:64, :], jnk[0:64, 0:64],
                         jnk[0:64, 64:64 + NQ], start=True, stop=True)

    # consolidated input loads, ordered so chunk-0 KV/Q start earliest
    nc.sync.dma_start(
        out=wkvr[:].rearrange("p (i c) -> p i c", i=8),
        in_=wkv.rearrange("(i p) c -> p i c", i=8))
    xtr3 = xtr[:].rearrange("p (i t) -> p i t", i=8)
    xT3 = xT.rearrange("(i p) t -> p i t", i=8)
    nc.sync.dma_start(out=xtr3[:, :, 0:NQ], in_=xT3[:, :, 0:NQ])
    nc.sync.dma_start(
        out=wqr[:].rearrange("p (i c) -> p i c", i=8),
        in_=wq.rearrange("(i p) c -> p i c", i=8))
    nc.sync.dma_start(out=cost[:, 0:NQ], in_=ctab[:, 0:NQ])
    nc.sync.dma_start(out=sint[:, 0:NQ], in_=stab[:, 0:NQ])
    # small tables ride the gpsimd queue; x chunks + rope tables on sync
    nc.gpsimd.dma_start(out=m2[:], in_=m2tab[:])
    nc.gpsimd.dma_start(out=umaskb[:], in_=umtab[:])
    nc.gpsimd.dma_start(out=bsel[:], in_=bseltab[:])
    nc.gpsimd.dma_start(
        out=wor[:].rearrange("p (j c) -> p j c", j=2),
        in_=wo.rearrange("(j p) c -> p j c", j=2))
    for cc in range(1, NCH):
        cs = slice(cc * NQ, (cc + 1) * NQ)
        nc.sync.dma_start(out=xtr3[:, :, cs], in_=xT3[:, :, cs])
        nc.sync.dma_start(out=cost[:, cs], in_=ctab[:, cs])
        nc.sync.dma_start(out=sint[:, cs], in_=stab[:, cs])

    def emit_kv(c):
        """KV proj chunk c; K rope into krot2[0:64,sl] + dup to [64:128];
        V parked transiently in krot2[64:128,sl], consumed into vaug +
        per-chunk sum redc[:,c] before the dup overwrites it."""
        sl = slice(c * NQ, (c + 1) * NQ)
        kvps = ps.tile([128, NQ], F32, tag="aux", bufs=2, name="kvps")
        for i in range(8):
            nc.tensor.matmul(kvps[:], wkvr[:, i * 128:(i + 1) * 128],
                             xtr[:, i * T + c * NQ:i * T + (c + 1) * NQ],
                             start=(i == 0), stop=(i == 7))
            if i in (1, 3, 5):
                yield
        kcp = stg.tile([128, NQ], DTMM, tag="pcp")
        with nc.allow_low_precision(reason="bf16 KV evac"):
            nc.vector.tensor_copy(kcp[0:64, :], kvps[0:64, :])
            nc.vector.tensor_copy(krot2[64:128, sl], kvps[64:128, :])
        yield
        for j in range(4):
            kt = c * 4 + j
            vtp = ps.tile([128, 64], DTMM, tag="aux", bufs=2, name="vtp")
            with nc.allow_low_precision(reason="bf16 PE transpose of V"):
                nc.tensor.transpose(vtp[:],
                                    krot2[64:128, kt * 128:(kt + 1) * 128],
                                    IDR[64:128, :])
                nc.vector.tensor_copy(vaug3[:, kt, 0:64], vtp[:])
            if j in (0, 1, 2):
                yield
        # per-chunk V sum (for the beyond-chunk analytic term)
        nc.vector.tensor_reduce(redc[64:128, c:c + 1], krot2[64:128, sl],
                                axis=mybir.AxisListType.X,
                                op=mybir.AluOpType.add)
        yield
        swp = stg.tile([128, NQ], DTMM, tag="swp")
        _half_swap(nc, swp, kcp, 0)
        t1 = stg.tile([128, NQ], DTMM, tag="t1")
        with nc.allow_low_precision(reason="bf16 K rope"):
            nc.vector.tensor_mul(t1[0:64, :], kcp[0:64, :], cost[0:64, sl])
            nc.vector.tensor_mul(swp[0:64, :], swp[0:64, :], sint[0:64, sl])
            nc.vector.tensor_add(krot2[0:64, sl], t1[0:64, :], swp[0:64, :])
        yield
        nc.gpsimd.dma_start(out=krot2[64:128, sl], in_=krot2[0:64, sl])

    def finish_suffix():
        # sfst[:, c] = sum_{c' > c} chunk_sum[c']  (both partition halves)
        nc.vector.memset(redc[64:128, 7:8], 0.0)
        nc.vector.tensor_copy(sfst[64:128, 3:4], redc[64:128, 7:8])
        for c in (2, 1, 0):
            nc.vector.tensor_add(sfst[64:128, c:c + 1],
                                 sfst[64:128, c + 1:c + 2],
                                 redc[64:128, c + 1:c + 2])
        nc.gpsimd.dma_start(out=sfst[0:64, :], in_=sfst[64:128, :])

    def emit_qproj(c, p):
        sweng = nc.gpsimd if c == 0 else nc.sync
        sl = slice(c * NQ, (c + 1) * NQ)
        qps = ps.tile([128, NQ], F32, tag="aux", bufs=2, name="qps")
        for i in range(8):
            nc.tensor.matmul(
                qps[:], wqr[:, i * 256 + p * 128: i * 256 + (p + 1) * 128],
                xtr[:, i * T + c * NQ:i * T + (c + 1) * NQ],
                start=(i == 0), stop=(i == 7))
            if i in (1, 3, 5):
                yield
        qcp = stg.tile([128, NQ], DTMM, tag="pcp", name="qcp")
        with nc.allow_low_precision(reason="bf16 Q evac"):
            nc.vector.tensor_copy(qcp[:], qps[:])
        swp = stg.tile([128, NQ], DTMM, tag="swp", name="swp")
        _half_swap(nc, swp, qcp, 0, eng=sweng)
        _half_swap(nc, swp, qcp, 64, eng=sweng)
        t1 = stg.tile([128, NQ], DTMM, tag="t1", name="t1")
        with nc.allow_low_precision(reason="bf16 Q rope"):
            nc.vector.tensor_mul(t1[:], qcp[:], cost[:, sl])
            nc.vector.tensor_mul(swp[:], swp[:], sint[:, sl])
            nc.vector.tensor_add(qrot[p][:, sl], t1[:], swp[:])

    COPYF = mybir.ActivationFunctionType.Copy

    def emit_yproj_unit(c, j):
        csl = slice(c * NQ, (c + 1) * NQ)
        jsl = slice(j * 128, (j + 1) * 128)
        if c == NCH - 1 and j % 2 == 1:
            # sg banks are free once the last exp has read them
            ysg = ps.tile([128, 2 * NQ], F32, tag="sg", bufs=2, name="ysg")
            yps = ysg[:, 0:NQ]
        else:
            yps = ps.tile([128, NQ], F32, tag="aux", bufs=2, name="yps")
        for p in range(2):
            nc.tensor.matmul(yps, wor[:, p * C + j * 128:p * C + (j + 1) * 128],
                             ostk[p][:, csl],
                             start=(p == 0), stop=(p == 1))
        ytmp = poolB.tile([128, NQ], DTMM, tag="ytmp", bufs=3, name="ytmp")
        with nc.allow_low_precision(reason="bf16 Y out"):
            if c == NCH - 1 and j % 2 == 0:
                nc.scalar.activation(ytmp[:], yps, COPYF)
            else:
                nc.vector.tensor_copy(ytmp[:], yps)
        nc.sync.dma_start(out=yT[jsl, csl], in_=ytmp[:])
        yield

    # two fill queues: P (KV/Q for upcoming chunks, priority) and Y.
    fillsP = []
    fillsY = []
    cur_fill = [None]
    allow_y = [False]

    def pump():
        while True:
            if cur_fill[0] is None:
                if fillsP:
                    cur_fill[0] = fillsP.pop(0)
                elif fillsY and allow_y[0]:
                    cur_fill[0] = fillsY.pop(0)
                else:
                    return
            try:
                next(cur_fill[0])
                return
            except StopIteration:
                cur_fill[0] = None

    def force(gen):
        if cur_fill[0] is gen:
            cur_fill[0] = None
        if gen in fillsP:
            fillsP.remove(gen)
        for _ in gen:
            pass

    def drain_fills():
        allow_y[0] = True
        while cur_fill[0] is not None or fillsP or fillsY:
            pump()

    # ---------- prelude: chunk 0 KV + Q; queue the rest ----------
    kv_gens = [emit_kv(c) for c in range(NCH)]
    q_gens = [[emit_qproj(c, p) for p in range(2)] for c in range(NCH)]
    force(kv_gens[0])
    force(q_gens[0][0])
    fillsP.extend([q_gens[0][1], kv_gens[1], q_gens[1][0], q_gens[1][1],
                   kv_gens[2], q_gens[2][0], q_gens[2][1],
                   kv_gens[3], q_gens[3][0], q_gens[3][1]])
    suffix_done = [False]

    LA = 3
    for c in range(NCH):
        allow_y[0] = c >= 2
        csl = slice(c * NQ, (c + 1) * NQ)
        for p in range(2):
            force(kv_gens[c])
            force(q_gens[c][p])
            nkt = 4 * (c + 1)
            opsE = ps.tile([128, NQ], F32, tag="opsE", bufs=1, name="opsE")
            opsO = ps.tile([128, NQ], F32, tag="opsO", bufs=1, name="opsO")
            pqs = {}
            for idx in range(nkt + LA):
                if idx < nkt:
                    kt = idx
                    dlt = max(0, (kt - 4 * c)) * 128 if kt >= 4 * c else 0
                    sg = ps.tile([128, 2 * NQ], F32, tag="sg", bufs=2, name="sg")
                    nc.tensor.matmul(sg[:, dlt:NQ],
                                     krot2[0:64, kt * 128:(kt + 1) * 128],
                                     qrot[p][0:64, c * NQ + dlt:(c + 1) * NQ],
                                     start=True, stop=True)
                    nc.tensor.matmul(sg[:, NQ + dlt:2 * NQ],
                                     krot2[64:128, kt * 128:(kt + 1) * 128],
                                     qrot[p][64:128, c * NQ + dlt:(c + 1) * NQ],
                                     start=True, stop=True, tile_position=(64, 0))
                    pq = poolB.tile([128, 2 * NQ], DTMM, tag="pq", bufs=8,
                                    name="pq")
                    if dlt:
                        sgv = sg[:].rearrange("p (h q) -> p h q", h=2)
                        pqv = pq[:].rearrange("p (h q) -> p h q", h=2)
                        nc.scalar.activation(pqv[:, :, dlt:NQ],
                                             sgv[:, :, dlt:NQ], EXP, scale=SCALE)
                    else:
                        nc.scalar.activation(pq[:], sg[:], EXP, scale=SCALE)
                    if kt >= 4 * c:     # diag: zero the in-tile strict-upper
                        pqv = pq[:].rearrange("p (h q) -> p h q", h=2)
                        m2v = m2[:].rearrange("p (h q) -> p h q", h=2)
                        with nc.allow_low_precision(reason="bf16 tri mask"):
                            nc.vector.tensor_mul(pqv[:, :, dlt:dlt + 128],
                                                 pqv[:, :, dlt:dlt + 128],
                                                 m2v[:])
                    pqs[kt] = pq
                if idx == 2:
                    # U-mask matmuls: analytic exp(0)=1 corrections for the
                    # strictly-masked in-chunk region; ones col fixes Z counts.
                    # j descends so the start=True write covers the full width
                    # (keeps the sim's PSUM zero-tracking happy; same sum).
                    for j in (3, 2, 1, 0):
                        kt_d = 4 * c + j
                        un = 128 * (j + 1)
                        nc.tensor.matmul(
                            opsE[0:65, 0:un],
                            vaug3[:, kt_d, :],
                            umaskb[:, j * NQ:j * NQ + un],
                            start=(j == 3), stop=False)
                        nc.tensor.matmul(
                            opsO[0:65, 0:un],
                            vaug3[:, kt_d, :],
                            umaskb[:, j * NQ:j * NQ + un],
                            start=(j == 3), stop=False)
                if idx >= LA:
                    kt = idx - LA
                    dlt = max(0, (kt - 4 * c)) * 128 if kt >= 4 * c else 0
                    pq = pqs.pop(kt)
                    nc.tensor.matmul(opsE[0:65, dlt:NQ],
                                     vaug3[:, kt, :],
                                     pq[:, dlt:NQ],
                                     start=False, stop=(kt == nkt - 1))
                    nc.tensor.matmul(opsO[0:65, dlt:NQ],
                                     vaug3[:, kt, :],
                                     pq[:, NQ + dlt:2 * NQ],
                                     start=False, stop=(kt == nkt - 1))
                if idx % 2 == 1:
                    pump()
            # tail: evacuate O+Z, spread-reciprocal Z, broadcast, normalize
            ocpE = poolB.tile([128, NQ], DTMM, tag="ocpE", bufs=2, name="ocpE")
            ocpO = poolB.tile([128, NQ], DTMM, tag="ocpO", bufs=2, name="ocpO")
            with nc.allow_low_precision(reason="bf16 O evac"):
                nc.vector.tensor_copy(ocpE[0:65, :], opsE[0:65])
                nc.vector.tensor_copy(ocpO[0:65, :], opsO[0:65])
            pump()
            zsp = poolB.tile([128, 16], F32, tag="zsp", bufs=2, name="zsp")
            nc.gpsimd.dma_start(
                out=zsp[:, 0:4],
                in_=ocpE[64:65, :].rearrange("p (a b) -> p a b", b=4))
            nc.gpsimd.dma_start(
                out=zsp[:, 4:8],
                in_=ocpO[64:65, :].rearrange("p (a b) -> p a b", b=4))
            cnt = float(T - (c + 1) * NQ)
            nc.vector.tensor_scalar_add(zsp[:, 8:16], zsp[:, 0:8], cnt)
            nc.vector.reciprocal(zsp[:, 0:8], zsp[:, 8:16])
            rz2 = poolB.tile([2, NQ], DTMM, tag="rz2", bufs=2, name="rz2")
            nc.gpsimd.dma_start(
                out=rz2[0:1, :].rearrange("p (a b) -> p a b", b=4),
                in_=zsp[:, 0:4])
            nc.gpsimd.dma_start(
                out=rz2[1:2, :].rearrange("p (a b) -> p a b", b=4),
                in_=zsp[:, 4:8])
            if not suffix_done[0]:
                # first tail: every chunk's V sum must exist for sfst
                for cc in range(1, NCH):
                    force(kv_gens[cc])
                finish_suffix()
                suffix_done[0] = True
            pump()
            rzp = ps.tile([128, NQ], F32, tag="aux", bufs=2, name="rzp")
            nc.tensor.matmul(rzp[:], bsel[:], rz2[:], start=True, stop=True)
            with nc.allow_low_precision(reason="bf16 normalized O"):
                nc.vector.scalar_tensor_tensor(
                    ostk[p][0:64, csl], ocpE[0:64, :], sfst[0:64, c:c + 1],
                    rzp[0:64, :], op0=mybir.AluOpType.add,
                    op1=mybir.AluOpType.mult)
                nc.vector.scalar_tensor_tensor(
                    ostk[p][64:128, csl], ocpO[0:64, :], sfst[0:64, c:c + 1],
                    rzp[64:128, :], op0=mybir.AluOpType.add,
                    op1=mybir.AluOpType.mult)
        for j in range(8):
            fillsY.append(emit_yproj_unit(c, j))
    drain_fills()


def _build(nrep=1, split=True):
    from contextlib import ExitStack
    nc = bass.Bass()
    xT = nc.declare_dram_parameter("xT", [C, T], DTMM, isOutput=False)
    wq = nc.declare_dram_parameter("wq", [C, HG * D], DTMM, isOutput=False)
    wkv = nc.declare_dram_parameter("wkv", [C, 2 * D], DTMM, isOutput=False)
    wo = nc.declare_dram_parameter("wo", [HG * D, C], DTMM, isOutput=False)
    ctab = nc.declare_dram_parameter("ctab", [128, T], DTMM, isOutput=False)
    stab = nc.declare_dram_parameter("stab", [128, T], DTMM, isOutput=False)
    m2tab = nc.declare_dram_parameter("m2tab", [128, 256], DTMM, isOutput=False)
    umtab = nc.declare_dram_parameter("umtab", [128, 4 * NQ], DTMM,
                                      isOutput=False)
    bseltab = nc.declare_dram_parameter("bseltab", [2, 128], DTMM,
                                        isOutput=False)
    yT = nc.declare_dram_parameter("yT", [C, T], DTMM, isOutput=True)

    with tile.TileContext(nc) as tc:
        for _ in range(nrep):
            with ExitStack() as ctx:
                _emit(nc, tc, ctx, xT, wq, wkv, wo, ctab, stab, m2tab, umtab,
                      bseltab, yT)
    if split:
        _split_waits(nc)
    return nc


def _host_inputs(x, Wq, Wk, Wv, Wo):
    perm = np.concatenate([np.arange(0, D, 2), np.arange(1, D, 2)])  # even-first
    inv_freq = 1.0 / (10000.0 ** (np.arange(0, D, 2, dtype=np.float64) / D))
    ang = np.arange(T, dtype=np.float64)[:, None] * inv_freq[None, :]
    cos = np.cos(ang).astype(np.float32).T      # (32, T)
    sin = np.sin(ang).astype(np.float32).T
    ctab = np.ascontiguousarray(np.tile(cos, (4, 1)).astype(NPMM))    # (128, T)
    stab = np.ascontiguousarray(
        np.concatenate([-sin, sin, -sin, sin], 0).astype(NPMM))
    pcol = np.arange(128)[:, None]
    t128 = np.arange(128)[None, :]
    tril01 = (pcol <= t128).astype(NPMM)
    m2tab = np.ascontiguousarray(np.concatenate([tril01, tril01], axis=1))
    f = np.arange(NQ)[None, :]
    umtab = np.ascontiguousarray(np.concatenate(
        [(pcol + i * 128 > f).astype(NPMM) for i in range(4)], axis=1))
    bseltab = np.zeros((2, 128), dtype=NPMM)
    bseltab[0, 0:64] = 1.0
    bseltab[1, 64:128] = 1.0

    xTb = [np.ascontiguousarray(x[b].T.astype(NPMM)) for b in range(B)]
    maps = []
    for core in range(8):
        b, g = core // 4, core % 4
        heads = [g + NKV * k for k in range(HG)]
        wq_cols = np.concatenate([h * D + perm for h in heads])
        wq_g = np.ascontiguousarray(Wq[:, wq_cols].astype(NPMM))
        wkv_g = np.ascontiguousarray(np.concatenate(
            [Wk[:, g * D + perm], Wv[:, g * D:(g + 1) * D]], axis=1).astype(NPMM))
        wo_rows = np.concatenate([np.arange(h * D, (h + 1) * D) for h in heads])
        wo_g = np.ascontiguousarray(Wo[wo_rows, :].astype(NPMM))
        maps.append({"xT": xTb[b], "wq": wq_g, "wkv": wkv_g, "wo": wo_g,
                     "ctab": ctab, "stab": stab, "m2tab": m2tab, "umtab": umtab,
                     "bseltab": bseltab})
    return maps


_CACHE = {}


def kernel(x, Wq, Wk, Wv, Wo):
    if "nc" not in _CACHE:
        _CACHE["nc"] = _build()
    nc = _CACHE["nc"]
    maps = _host_inputs(np.asarray(x, np.float32), np.asarray(Wq, np.float32),
                        np.asarray(Wk, np.float32), np.asarray(Wv, np.float32),
                        np.asarray(Wo, np.float32))
    trace = bool(int(os.environ.get("BASSKERNEL_TRACE", "0")))
    res = run_bass_kernel_spmd(nc, maps, list(range(8)), trace=trace)
    if trace and res.exec_time_ns is not None:
        print(f"HW exec time: {res.exec_time_ns} ns")
    out = np.zeros((B, T, C), dtype=np.float32)
    for core in range(8):
        out[core // 4] += res.results[core]["yT"].astype(np.float32).T
    return out
